# revision 1
# baseline (speedup 1.0000x reference)
"""Trainium2 Bass kernel for the spiking-network simulation (nn_AbstractModel_70394513981830).

Per step: s' = sigmoid(segment_sum(W * s[src], dst) - 1).  300 sequential steps,
last 200 recorded.  Strategy (8 NeuronCores, SPMD):

- dst-sharded: NC k owns dst nodes [k*12500, (k+1)*12500); within an NC, Q7 core
  c owns local dsts [c*1563, (c+1)*1563).
- state table SBUF [128, NTAB] f32, table[p, i] = s_rho[16*i + p%16]  (state
  replicated once per 16-partition lane group, so every Q7 core can gather any
  node via GPSIMD ap_gather with int16 indices).
- per step, per slab of 4096 edge slots: ap_gather fetches s[src] (value lands at
  partition 16c + rho(src)%16 of core c's slot column), a static weight-mask
  multiply kills the 15 garbage lanes, an 8-slot free-dim reduce and a
  block-diagonal ones16 matmul (lane sum) compact the products, and a prefix-sum
  over 8-chunks + endpoint-subtract extraction produce per-dst sums (edge streams
  are dst-sorted with runs padded to multiples of 8, never crossing slabs).
- sigmoid on ACT, shard exchange via AllGather, table rebuilt by affine DMA,
  raster rows written via indirect DMA driven by an on-chip step counter.
"""

import numpy as np

LAST_RUN_NS = 0
N_NODES = 100_000
N_EDGES = 6_400_000
THETA = 1.0
NC = 8                      # NeuronCores
D_NC = N_NODES // NC        # dsts per NeuronCore (12500)
D_CORE = 1563               # dsts per Q7 core (8*1563 = 12504 >= 12500)
D_PAD = NC * D_CORE         # padded shard size 12504
NTAB = (NC * D_PAD) // 16   # table free size 6252
SLAB = 4096                 # edge slots per slab
CHUNK = 8                   # run padding granularity
NXT_PAD = 3200              # extraction idxs per core (2*1563=3126 padded)


def _preprocess(x, W, edge_index):
    """Build per-NeuronCore static streams. Returns (common, per_core list)."""
    src = np.asarray(edge_index[0], dtype=np.int64)
    dst = np.asarray(edge_index[1], dtype=np.int64)
    W = np.asarray(W, dtype=np.float32)
    k_of = dst // D_NC
    rho_src = (src // D_NC) * D_PAD + (src % D_NC)   # node -> padded position

    streams = []
    for k in range(NC):
        sel = np.nonzero(k_of == k)[0]
        dl = (dst[sel] % D_NC).astype(np.int64)
        c = dl // D_CORE
        rank = dl % D_CORE
        rs = rho_src[sel]
        w = W[sel]
        core_streams = []
        for cc in range(NC):
            m = np.nonzero(c == cc)[0]
            order = np.argsort(rank[m], kind="stable")
            m = m[order]
            rk = rank[m].astype(np.int64)
            cnts = np.bincount(rk, minlength=D_CORE).astype(np.int64)
            core_streams.append((rk, rs[m].astype(np.int64), w[m], cnts))
        streams.append(core_streams)

    def pack(cnts):
        """Slot start per rank.  Each slab starts with 8 pad slots; runs are
        padded to multiples of 8 and never cross a slab boundary."""
        a = np.zeros(D_CORE, dtype=np.int64)
        pos = CHUNK
        slab = 0
        for r in range(D_CORE):
            L = int((cnts[r] + CHUNK - 1) // CHUNK * CHUNK)
            if pos + L > SLAB:
                slab += 1
                pos = CHUNK
            a[r] = slab * SLAB + pos
            pos += L
        return a, slab + 1

    packs, slabs_needed = [], 0
    for k in range(NC):
        row = []
        for cc in range(NC):
            a, ns = pack(streams[k][cc][3])
            row.append(a)
            slabs_needed = max(slabs_needed, ns)
        packs.append(row)

    NSLAB = slabs_needed
    NP8 = NSLAB * (SLAB // CHUNK)
    assert NP8 <= 32768 and NTAB <= 32768

    per_core = []
    for k in range(NC):
        gidx = np.zeros((128, NSLAB * (SLAB // 16)), dtype=np.int16)
        import ml_dtypes
        wtil = np.zeros((NSLAB, 128, SLAB), dtype=ml_dtypes.bfloat16)
        xidx = np.zeros((128, NXT_PAD // 16), dtype=np.int16)
        for cc in range(NC):
            rk, rs, w, cnts = streams[k][cc]
            a = packs[k][cc]
            starts = np.concatenate([[0], np.cumsum(cnts)[:-1]])
            occ = np.arange(len(rk)) - starts[rk]
            slot = a[rk] + occ
            s_slab = slot // SLAB
            jj = slot % SLAB
            lane = rs % 16
            gval = (rs // 16).astype(np.int16)
            gidx[16 * cc + (jj % 16), s_slab * (SLAB // 16) + jj // 16] = gval
            wtil[s_slab, 16 * cc + lane, jj] = w.astype(ml_dtypes.bfloat16)
            L = (cnts + CHUNK - 1) // CHUNK * CHUNK
            b_chunk = ((a + L) // CHUNK - 1)
            a_chunk = (a // CHUNK - 1)
            empty = cnts == 0
            b_chunk[empty] = 0
            a_chunk[empty] = 0
            pairs = np.empty(2 * D_CORE, dtype=np.int16)
            pairs[0::2] = b_chunk.astype(np.int16)
            pairs[1::2] = a_chunk.astype(np.int16)
            pidx = np.arange(2 * D_CORE)
            xidx[16 * cc + (pidx % 16), pidx // 16] = pairs
        per_core.append({"gidx": gidx, "wtil": wtil, "xidx": xidx})

    s0 = np.asarray(x, dtype=np.float32).reshape(-1)
    s_rho = np.zeros(NC * D_PAD, dtype=np.float32)
    for k in range(NC):
        s_rho[k * D_PAD:k * D_PAD + D_NC] = s0[k * D_NC:(k + 1) * D_NC]
    table0 = np.zeros((128, NTAB), dtype=np.float32)
    for l in range(16):
        table0[l::16, :] = np.broadcast_to(s_rho[l::16][None, :], (8, NTAB))
    return {"NSLAB": NSLAB, "NP8": NP8, "table0": table0}, per_core


def _step_body(nc, s_in, gidx_in, wtil_in, xidx_in, ones16_in, bias_in, NSLAB, NP8):
    import concourse.mybir as mybir
    import concourse.tile as tile

    f32 = mybir.dt.float32
    bf16 = mybir.dt.bfloat16
    AF = mybir.ActivationFunctionType
    ALU = mybir.AluOpType
    AX = mybir.AxisListType
    NCHUNK = SLAB // CHUNK
    if True:
        shard = nc.dram_tensor("shard_out", [1, D_PAD], f32, kind="ExternalOutput")
        with tile.TileContext(nc) as tc:
            with tc.tile_pool(name="pool", bufs=1) as pool, \
                 tc.tile_pool(name="gpool", bufs=2) as gpool, \
                 tc.tile_pool(name="ppool", bufs=2, space="PSUM") as ppool:
                t_tab = pool.tile([128, NTAB], f32)
                t_gidx = pool.tile([128, NSLAB * (SLAB // 16)], mybir.dt.int16)
                t_xidx = pool.tile([128, NXT_PAD // 16], mybir.dt.int16)
                t_ones = pool.tile([128, 128], f32)
                t_bias = pool.tile([128, 1], f32)
                t_p8 = pool.tile([128, NP8], f32)
                t_ppa = pool.tile([128, 256 + NCHUNK], f32)
                t_ppb = pool.tile([128, 256 + NCHUNK], f32)
                t_ebuf = pool.tile([128, NXT_PAD], f32)
                t_diff = pool.tile([128, NXT_PAD // 2], f32)
                t_snew = pool.tile([128, NXT_PAD // 2], f32)
                t_stage = pool.tile([8, D_CORE], f32)

                cc_flat = s_in[:].rearrange("a b -> (a b)").rearrange("(i l) -> l i", l=16)
                for g in range(8):
                    nc.sync.dma_start(t_tab[16 * g:16 * g + 16, :], cc_flat)
                nc.sync.dma_start(t_gidx[:], gidx_in[0])
                nc.sync.dma_start(t_xidx[:], xidx_in[0])
                nc.sync.dma_start(t_ones[:], ones16_in[:])
                nc.sync.dma_start(t_bias[:], bias_in[:])
                nc.vector.memset(t_ppa[:], 0.0)
                nc.vector.memset(t_ppb[:], 0.0)

                for s in range(NSLAB):
                    t_w = gpool.tile([128, SLAB], bf16, tag="w")
                    t_g = gpool.tile([128, SLAB], f32, tag="g")
                    t_v = gpool.tile([128, SLAB], bf16, tag="v")
                    t_r8 = gpool.tile([128, NCHUNK], f32, tag="r8")
                    nc.sync.dma_start(t_w[:], wtil_in[0, s])
                    nc.gpsimd.ap_gather(
                        t_g[:], t_tab[:],
                        t_gidx[:, s * (SLAB // 16):(s + 1) * (SLAB // 16)],
                        channels=128, num_elems=NTAB, d=1, num_idxs=SLAB,
                    )
                    nc.vector.tensor_tensor(t_v[:], t_g[:], t_w[:], ALU.mult)
                    nc.vector.tensor_reduce(
                        t_r8[:], t_v[:].rearrange("p (a b) -> p a b", b=CHUNK),
                        axis=AX.X, op=ALU.add,
                    )
                    t_ps = ppool.tile([128, NCHUNK], f32, tag="ps")
                    nc.tensor.matmul(t_ps[:], t_ones[:], t_r8[:])
                    nc.vector.tensor_copy(t_ppa[:, 256:256 + NCHUNK], t_ps[:])
                    cur, nxt = t_ppa, t_ppb
                    for r in range(9):
                        sh = 1 << r
                        dst_ap = (t_p8[:, s * NCHUNK:(s + 1) * NCHUNK]
                                  if r == 8 else nxt[:, 256:256 + NCHUNK])
                        nc.vector.tensor_tensor(
                            dst_ap, cur[:, 256:256 + NCHUNK],
                            cur[:, 256 - sh:256 - sh + NCHUNK], ALU.add)
                        cur, nxt = nxt, cur

                nc.gpsimd.ap_gather(
                    t_ebuf[:], t_p8[:], t_xidx[:],
                    channels=128, num_elems=NP8, d=1, num_idxs=NXT_PAD,
                )
                ev = t_ebuf[:].rearrange("p (a two) -> p a two", two=2)
                nc.vector.tensor_tensor(t_diff[:], ev[:, :, 0], ev[:, :, 1], ALU.subtract)
                nc.scalar.activation(t_snew[:], t_diff[:], AF.Sigmoid, bias=t_bias[:])
                nc.sync.dma_start(
                    t_stage[:],
                    t_snew[:].rearrange("(a b) f -> a b f", b=16)[:, 0, :D_CORE],
                )
                nc.sync.dma_start(shard[:], t_stage[:])
    return shard


def _make_step(NSLAB, NP8):
    from functools import partial
    from concourse.bass2jax import bass_jit

    def step_fun(nc, s_in, gidx_in, wtil_in, xidx_in, ones16_in, bias_in):
        return _step_body(nc, s_in, gidx_in, wtil_in, xidx_in, ones16_in,
                          bias_in, NSLAB, NP8)

    return bass_jit(step_fun)


def kernel(x, W, edge_index, n_steps, equilibration_steps):
    import jax
    import jax.numpy as jnp
    from jax.sharding import Mesh, NamedSharding, PartitionSpec as P
    from concourse.bass2jax import bass_shard_map
    import time as _t

    n_steps = int(n_steps)
    equilibration_steps = int(equilibration_steps)
    total = n_steps + equilibration_steps

    _t0 = _t.perf_counter()
    common, per_core = _preprocess(x, W, edge_index)
    NSLAB, NP8 = common["NSLAB"], common["NP8"]
    print(f"[kernel] preprocess {_t.perf_counter()-_t0:.1f}s NSLAB={NSLAB}", flush=True)

    step = _make_step(NSLAB, NP8)

    ones16 = np.zeros((128, 128), dtype=np.float32)
    for g in range(8):
        ones16[16 * g:16 * g + 16, 16 * g:16 * g + 16] = 1.0
    bias = np.full((128, 1), -THETA, dtype=np.float32)

    devices = np.array(jax.devices()[:NC])
    mesh = Mesh(devices, ("d",))
    sh_d = NamedSharding(mesh, P("d"))
    sh_r = NamedSharding(mesh, P())

    import ml_dtypes
    gidx_all = jax.device_put(np.stack([pc["gidx"] for pc in per_core]), sh_d)
    wtil_all = jax.device_put(
        np.stack([pc["wtil"] for pc in per_core]).view(ml_dtypes.bfloat16), sh_d)
    xidx_all = jax.device_put(np.stack([pc["xidx"] for pc in per_core]), sh_d)
    ones_r = jax.device_put(ones16, sh_r)
    bias_r = jax.device_put(bias, sh_r)

    s0 = np.asarray(x, dtype=np.float32).reshape(-1)
    s_rho0 = np.zeros((1, NC * D_PAD), dtype=np.float32)
    for k in range(NC):
        s_rho0[0, k * D_PAD:k * D_PAD + D_NC] = s0[k * D_NC:(k + 1) * D_NC]
    s_flat = jax.device_put(s_rho0, sh_r)

    step_sm = bass_shard_map(
        step,
        mesh=mesh,
        in_specs=(P(), P("d"), P("d"), P("d"), P(), P()),
        out_specs=P("d"),
    )

    @jax.jit
    def regather(shards):          # [8, D_PAD] sharded -> [1, 100032] replicated
        return jax.lax.with_sharding_constraint(
            shards.reshape(1, NC * D_PAD), sh_r)

    global LAST_RUN_NS
    _t0 = _t.perf_counter()
    raster_dev = []
    for t in range(total):
        shards = step_sm(s_flat, gidx_all, wtil_all, xidx_all, ones_r, bias_r)
        s_flat = regather(shards)
        if t >= equilibration_steps:
            raster_dev.append(shards)
    shards.block_until_ready()
    dt = _t.perf_counter() - _t0
    LAST_RUN_NS = int(dt * 1e9)
    print(f"[kernel] run {dt:.2f}s ({dt/total*1e3:.2f} ms/step)", flush=True)

    _t0 = _t.perf_counter()
    raster_np = jax.device_get(raster_dev)     # list of [8, D_PAD]
    arr = np.stack(raster_np)                  # [n_steps, 8, D_PAD]
    out = np.ascontiguousarray(
        arr[:, :, :D_NC].reshape(n_steps, NC * D_NC))
    print(f"[kernel] fetch {_t.perf_counter()-_t0:.2f}s", flush=True)
    return out



# revision 6
# speedup vs baseline: 1.3307x; 1.3307x over previous
"""Trainium2 Bass kernel for nn_AbstractModel_70394513981830 (spiking network).

Per step: s' = sigmoid(segment_sum(W * s[src], dst) - 1), 300 sequential steps,
last 200 recorded. 8 NeuronCores, dst-sharded (NC k owns dsts [12500k,12500k+12500)).

Single fused kernel launch runs all 300 steps (unrolled; one straight-line
AllGather per step). Per-step pipeline on each NC (its 800k edges):

1. s_all [128, 848] bf16 holds the full state: node n = 12500*k + d lives at
   partition d%128, column 106*k + d//128; columns 98..105 of each sender block
   are extension copies of "overloaded" nodes (see below).
2. Expansion multiply (DVE): per-edge products P1[p, occ*848 + j] =
   W_exp[p, occ*848 + j] * s_all[p, j] via a stride-0 broadcast view. Each
   node-column j has a run of R=16 occurrence slots; a (node, receiver) pair
   with >16 in-edges spills into the node's extension column (16 more).
3. Route each product to the partition owning its dst in 3 stages:
   a. local_scatter (per-partition idxs) P1 -> R1 at position 128*b + p_dst
      (b = per (src-partition, dst-partition) bucket fill), in windows of 2047.
   b. full 128x128 block transposes of R1: DVE stream_transpose (32x32) + 16
      block-grid DMAs => R2[p_dst, 128*b + q_src] = R1[q_src, 128*b + p_dst].
   c. local_scatter R2 -> R3: per-partition dst runs, padded to multiples of 8.
4. reduce8 (f32) -> chunk sums; bf16; local_scatter chunks into per-dst
   16-slot bins; reduce16 -> per-dst sums [128, 98] f32; sigmoid(x-1) -> bf16.
5. local_scatter fills the 8 extension columns; shard [128, 106] -> DRAM;
   AllGather; 8 DMAs rebuild s_all; raster row written per step.
"""

import numpy as np

LAST_RUN_NS = 0

N_NODES = 100_000
N_EDGES = 6_400_000
THETA = 1.0
NC = 8
NCD = N_NODES // NC          # 12500 dsts / nodes per NC
COLS = 98                    # ceil(12500 / 128) node columns per NC
EXT = 12                     # extension columns per NC
CPN = COLS + EXT             # 106 columns per NC block in s_all
NODECOLS = NC * CPN          # 848
R = 16                       # occurrence slots per node column
M1 = R * NODECOLS            # 13568 expansion slots per partition
WIN = 2046                   # local_scatter out window size (even)
BINS = 16                    # chunk bins per dst
MAXRANK = 98                 # dst ranks per partition


def _group_rank(keys):
    """Rank of each element within its equal-key group (keys int64 1-D)."""
    order = np.argsort(keys, kind="stable")
    sk = keys[order]
    first = np.r_[0, np.flatnonzero(np.diff(sk)) + 1]
    grp_start = np.zeros(len(sk), dtype=np.int64)
    grp_start[first] = first
    np.maximum.accumulate(grp_start, out=grp_start)
    rank_sorted = np.arange(len(sk)) - grp_start
    rank = np.empty(len(sk), dtype=np.int64)
    rank[order] = rank_sorted
    return rank


def _preprocess(x, W, edge_index):
    import ml_dtypes

    src = np.asarray(edge_index[0], dtype=np.int64)
    dst = np.asarray(edge_index[1], dtype=np.int64)
    W = np.asarray(W, dtype=np.float32)
    s0 = np.asarray(x, dtype=np.float32).reshape(-1)

    k_r = dst // NCD                       # receiver NC per edge
    ks = src // NCD                        # sender NC per src node
    ds = src % NCD
    q_s = ds % 128                         # src partition
    base_j = CPN * ks + ds // 128          # base node column

    # (node, receiver) in-multiplicity; extension column assignment (global)
    cnt = np.bincount(src * NC + k_r, minlength=N_NODES * NC).reshape(N_NODES, NC)
    assert cnt.max() <= 2 * R, f"node multiplicity {cnt.max()} > {2*R}"
    over = (cnt > R).any(axis=1)           # nodes needing an extension column
    node_ks = np.arange(N_NODES) // NCD
    node_q = (np.arange(N_NODES) % NCD) % 128
    ext_col = np.full(N_NODES, -1, dtype=np.int64)
    ext_slot = np.full(N_NODES, -1, dtype=np.int64)
    ov_idx = np.flatnonzero(over)
    slot = _group_rank(node_ks[ov_idx] * 128 + node_q[ov_idx])
    assert len(slot) == 0 or slot.max() < EXT, \
        f"extension slots per (sender,partition) exceed {EXT}: {slot.max()+1}"
    ext_slot[ov_idx] = slot
    ext_col[ov_idx] = CPN * node_ks[ov_idx] + COLS + slot

    per_core = []
    B_all, L3_all = [], []
    cores_tmp = []
    for k in range(NC):
        sel = np.flatnonzero(k_r == k)
        e_src = src[sel]
        e_w = W[sel]
        dl = dst[sel] - NCD * k
        p_d = dl % 128
        r_d = dl // 128
        eq = q_s[sel]

        occ = _group_rank(e_src)
        use_ext = occ >= R
        j = np.where(use_ext, ext_col[e_src], base_j[sel])
        occ_slot = np.where(use_ext, occ - R, occ)
        assert (j >= 0).all() and (occ_slot < R).all()
        m1 = occ_slot * NODECOLS + j       # P1 slot (partition eq)

        b = _group_rank(eq * 128 + p_d)    # bucket fill (q_src, p_dst)
        B = int(b.max()) + 1

        # dst runs (per dst-partition), padded to multiples of 8
        deg = np.bincount(p_d * MAXRANK + r_d,
                          minlength=128 * MAXRANK).reshape(128, MAXRANK)
        runlen = (deg + 7) // 8 * 8
        starts = np.zeros_like(runlen)
        starts[:, 1:] = np.cumsum(runlen, axis=1)[:, :-1]
        L3 = int((starts[:, -1] + runlen[:, -1]).max())
        within = _group_rank(p_d * MAXRANK + r_d)
        slot3 = starts[p_d, r_d] + within

        cores_tmp.append(dict(
            e_src=e_src, e_w=e_w, p_d=p_d, r_d=r_d, eq=eq, m1=m1, b=b,
            slot3=slot3, deg=deg, runlen=runlen, starts=starts,
        ))
        B_all.append(B)
        L3_all.append(L3)

    B = max(B_all)
    M2 = 128 * B                           # routed-layout size
    W1 = -(-M2 // WIN)                     # stage-a windows
    L3 = -(-max(L3_all) // 16) * 16        # multiple of 16 (even NCHUNK)
    W3 = -(-L3 // WIN)
    NCHUNK = L3 // 8
    assert M1 % 2 == 0 and M2 % 2 == 0 and NCHUNK % 2 == 0

    for k in range(NC):
        t = cores_tmp[k]
        eq, m1, p_d, b = t["eq"], t["m1"], t["p_d"], t["b"]
        r1pos = 128 * b + p_d
        idx1 = np.full((W1, 128, M1), -1, dtype=np.int16)
        w = r1pos // WIN
        idx1[w, eq, m1] = (r1pos - w * WIN).astype(np.int16)

        m2 = 128 * b + eq                  # position after transpose (part p_d)
        slot3 = t["slot3"]
        idx3 = np.full((W3, 128, M2), -1, dtype=np.int16)
        w = slot3 // WIN
        idx3[w, p_d, m2] = (slot3 - w * WIN).astype(np.int16)

        # chunk -> bin mapping
        runlen, starts = t["runlen"], t["starts"]
        idx_bins = np.full((128, NCHUNK), -1, dtype=np.int16)
        for p in range(128):
            for r in range(MAXRANK):
                rl = runlen[p, r]
                if rl == 0:
                    continue
                c0 = starts[p, r] // 8
                nch = rl // 8
                assert nch <= BINS
                idx_bins[p, c0:c0 + nch] = np.arange(
                    BINS * r, BINS * r + nch, dtype=np.int16)

        w_exp = np.zeros((128, M1), dtype=ml_dtypes.bfloat16)
        w_exp[eq, m1] = t["e_w"].astype(ml_dtypes.bfloat16)

        # sender-side extension fill: this core's own overloaded nodes
        extf = np.full((128, COLS), -1, dtype=np.int16)
        own = np.arange(NCD * k, NCD * (k + 1))
        ov = over[own]
        d_own = np.arange(NCD)[ov]
        extf[d_own % 128, d_own // 128] = ext_slot[own[ov]].astype(np.int16)

        per_core.append(dict(w_exp=w_exp, idx1=idx1, idx3=idx3,
                             idx_bins=idx_bins, extf=extf))

    # initial s_all (same for all cores)
    s_pad = np.zeros(128 * NODECOLS, dtype=np.float32).reshape(128, NODECOLS)
    n = np.arange(N_NODES)
    s_pad[(n % NCD) % 128, CPN * (n // NCD) + (n % NCD) // 128] = s0
    s_pad[node_q[ov_idx], ext_col[ov_idx]] = s0[ov_idx]
    s_all0 = s_pad.astype(ml_dtypes.bfloat16)

    meta = dict(B=B, M2=M2, W1=W1, W3=W3, L3=L3, NCHUNK=NCHUNK, s_all0=s_all0)
    return meta, per_core


def _np_step(meta, per_core, s_all):
    """Numpy simulation of one device step. s_all [128, 848] bf16 (all cores
    share it). Returns (new s_all bf16, shards list of [128, COLS] bf16)."""
    import ml_dtypes

    B, M2, W1, W3 = meta["B"], meta["M2"], meta["W1"], meta["W3"]
    L3, NCHUNK = meta["L3"], meta["NCHUNK"]
    shards = []
    for k in range(NC):
        pc = per_core[k]
        s_view = np.tile(s_all[:, None, :], (1, R, 1)).reshape(128, M1)
        p1 = (pc["w_exp"].astype(np.float32) * s_view.astype(np.float32)
              ).astype(ml_dtypes.bfloat16)
        r1 = np.zeros((128, W1 * WIN), dtype=ml_dtypes.bfloat16)
        for w in range(W1):
            ii = pc["idx1"][w]
            pmask, mmask = np.nonzero(ii >= 0)
            r1[pmask, w * WIN + ii[pmask, mmask]] = p1[pmask, mmask]
        r2 = np.zeros((128, M2), dtype=ml_dtypes.bfloat16)
        blk = r1[:, :M2].reshape(128, B, 128)
        r2 = blk.transpose(2, 1, 0).reshape(128, M2)
        r3 = np.zeros((128, L3), dtype=ml_dtypes.bfloat16)
        for w in range(W3):
            ii = pc["idx3"][w]
            pmask, mmask = np.nonzero(ii >= 0)
            r3[pmask, w * WIN + ii[pmask, mmask]] = r2[pmask, mmask]
        chunks = r3.astype(np.float32).reshape(128, NCHUNK, 8).sum(-1)
        chunks_bf = chunks.astype(ml_dtypes.bfloat16)
        bins = np.zeros((128, MAXRANK * BINS), dtype=ml_dtypes.bfloat16)
        ib = pc["idx_bins"]
        pmask, cmask = np.nonzero(ib >= 0)
        bins[pmask, ib[pmask, cmask]] = chunks_bf[pmask, cmask]
        sums = bins.astype(np.float32).reshape(128, MAXRANK, BINS).sum(-1)
        snew = (1.0 / (1.0 + np.exp(-(sums - THETA)))).astype(ml_dtypes.bfloat16)
        shards.append(snew)

    new_sall = np.zeros_like(s_all)
    for k in range(NC):
        pc = per_core[k]
        blkk = np.zeros((128, CPN), dtype=ml_dtypes.bfloat16)
        blkk[:, :COLS] = shards[k]
        ef = pc["extf"]
        pmask, cmask = np.nonzero(ef >= 0)
        blkk[pmask, COLS + ef[pmask, cmask]] = shards[k][pmask, cmask]
        new_sall[:, CPN * k:CPN * (k + 1)] = blkk
    return new_sall, shards


def _sall_to_s(s_all):
    """Extract the flat [N_NODES] f32 state from an s_all table."""
    n = np.arange(N_NODES)
    return s_all[(n % NCD) % 128,
                 CPN * (n // NCD) + (n % NCD) // 128].astype(np.float32)


def _build(meta, steps):
    """Build the unrolled bass program (shared by all 8 cores)."""
    import concourse.bass as bass
    import concourse.bacc as bacc
    import concourse.mybir as mybir
    import concourse.tile as tile

    f32 = mybir.dt.float32
    bf16 = mybir.dt.bfloat16
    i16 = mybir.dt.int16
    AF = mybir.ActivationFunctionType
    ALU = mybir.AluOpType
    AX = mybir.AxisListType

    B, M2, W1, W3 = meta["B"], meta["M2"], meta["W1"], meta["W3"]
    L3, NCHUNK = meta["L3"], meta["NCHUNK"]

    nc = bacc.Bacc(num_devices=NC)
    g_wexp = nc.dram_tensor("w_exp", [128, M1], bf16, kind="ExternalInput")
    g_idx1 = nc.dram_tensor("idx1", [W1, 128, M1], i16, kind="ExternalInput")
    g_idx3 = nc.dram_tensor("idx3", [W3, 128, M2], i16, kind="ExternalInput")
    g_bins = nc.dram_tensor("idx_bins", [128, NCHUNK], i16, kind="ExternalInput")
    g_extf = nc.dram_tensor("extf", [128, COLS], i16, kind="ExternalInput")
    g_sall0 = nc.dram_tensor("s_all0", [128, NODECOLS], bf16, kind="ExternalInput")
    raster = nc.dram_tensor("raster", [steps, 128, COLS], bf16,
                            kind="ExternalOutput")
    cc_in = nc.dram_tensor("cc_in", [128, CPN], bf16)
    cc_out = nc.dram_tensor("cc_out", [NC * 128, CPN], bf16, addr_space="Shared")

    with tile.TileContext(nc) as tc:
        with tc.tile_pool(name="res", bufs=1) as res, \
             tc.tile_pool(name="big", bufs=1) as big, \
             tc.tile_pool(name="idx", bufs=2) as idxp:
            t_wexp = res.tile([128, M1], bf16)
            t_sall = res.tile([128, NODECOLS], bf16)
            t_bins_i = res.tile([128, NCHUNK], i16)
            t_extf = res.tile([128, COLS], i16)
            t_bias = res.tile([128, 1], f32)
            nc.vector.memset(t_bias[:], -THETA)
            nc.sync.dma_start(t_wexp[:], g_wexp[:])
            nc.sync.dma_start(t_sall[:], g_sall0[:])
            nc.sync.dma_start(t_bins_i[:], g_bins[:])
            nc.sync.dma_start(t_extf[:], g_extf[:])

            for t in range(steps):
                # 1. expansion multiply
                t_p1 = big.tile([128, M1], bf16, tag="p1t1")
                s_b = t_sall[:].unsqueeze(1).broadcast_to([128, R, NODECOLS])
                nc.vector.tensor_tensor(
                    t_p1[:].rearrange("p (o j) -> p o j", o=R),
                    t_wexp[:].rearrange("p (o j) -> p o j", o=R),
                    s_b, ALU.mult)

                # 2a. route scatter 1 (P1 -> R1)
                t_r1 = big.tile([128, W1 * WIN], bf16, tag="r1r2")
                for w in range(W1):
                    t_i1 = idxp.tile([128, M1], i16, tag="idx")
                    nc.sync.dma_start(t_i1[:], g_idx1[w])
                    nc.gpsimd.local_scatter(
                        t_r1[:, w * WIN:(w + 1) * WIN], t_p1[:], t_i1[:],
                        channels=128, num_elems=WIN, num_idxs=M1)

                # 2b. 128-block transposes
                t_t1 = big.tile([128, M2], bf16, tag="p1t1")
                nc.vector.transpose(t_t1[:], t_r1[:, :M2])
                t_r2 = big.tile([128, W1 * WIN], bf16, tag="r1r2")
                for a in range(4):
                    for d in range(4):
                        nc.sync.dma_start(
                            t_r2[32 * a:32 * a + 32, :M2]
                            .rearrange("p (b d w) -> p b d w", d=4, w=32)
                            [:, :, d, :],
                            t_t1[32 * d:32 * d + 32, :]
                            .rearrange("p (b a w) -> p b a w", a=4, w=32)
                            [:, :, a, :])

                # 2c. route scatter 2 (R2 -> R3 dst runs)
                t_r3 = big.tile([128, L3], bf16, tag="r3")
                for w in range(W3):
                    t_i3 = idxp.tile([128, M2], i16, tag="idx")
                    nc.sync.dma_start(t_i3[:], g_idx3[w])
                    nc.gpsimd.local_scatter(
                        t_r3[:, w * WIN:min((w + 1) * WIN, L3)], t_r2[:, :M2],
                        t_i3[:], channels=128,
                        num_elems=min(WIN, L3 - w * WIN), num_idxs=M2)

                # 3. segment sums
                t_ch = big.tile([128, NCHUNK], f32, tag="ch")
                nc.vector.tensor_reduce(
                    t_ch[:], t_r3[:].rearrange("p (c e) -> p c e", e=8),
                    axis=AX.X, op=ALU.add)
                t_chb = big.tile([128, NCHUNK], bf16, tag="chb")
                nc.vector.tensor_copy(t_chb[:], t_ch[:])
                t_bins = big.tile([128, MAXRANK * BINS], bf16, tag="bins")
                nc.gpsimd.local_scatter(
                    t_bins[:], t_chb[:], t_bins_i[:],
                    channels=128, num_elems=MAXRANK * BINS, num_idxs=NCHUNK)
                t_sum = big.tile([128, MAXRANK], f32, tag="sum")
                nc.vector.tensor_reduce(
                    t_sum[:], t_bins[:].rearrange("p (r e) -> p r e", e=BINS),
                    axis=AX.X, op=ALU.add)

                # 4. sigmoid + extension fill -> shard
                t_shard = big.tile([128, CPN], bf16, tag="shard")
                nc.scalar.activation(t_shard[:, :COLS], t_sum[:], AF.Sigmoid,
                                     bias=t_bias[:])
                nc.gpsimd.local_scatter(
                    t_shard[:, COLS:CPN], t_shard[:, :COLS], t_extf[:],
                    channels=128, num_elems=EXT, num_idxs=COLS)

                # 5. raster + exchange + s_all rebuild
                nc.sync.dma_start(raster[t], t_shard[:, :COLS])
                nc.sync.dma_start(cc_in[:], t_shard[:])
                nc.gpsimd.collective_compute(
                    "AllGather", ALU.bypass,
                    replica_groups=[list(range(NC))],
                    ins=[cc_in[:]], outs=[cc_out[:]])
                for k in range(NC):
                    nc.sync.dma_start(
                        t_sall[:, CPN * k:CPN * (k + 1)],
                        cc_out[128 * k:128 * (k + 1), :])
    nc.finalize()
    return nc


def kernel(x, W, edge_index, n_steps, equilibration_steps):
    import time as _t
    from concourse.bass_utils import run_bass_kernel_spmd

    global LAST_RUN_NS
    n_steps = int(n_steps)
    equilibration_steps = int(equilibration_steps)
    total = n_steps + equilibration_steps

    t0 = _t.perf_counter()
    meta, per_core = _preprocess(x, W, edge_index)
    print(f"[kernel] preprocess {_t.perf_counter()-t0:.1f}s "
          f"B={meta['B']} W1={meta['W1']} L3={meta['L3']}", flush=True)

    t0 = _t.perf_counter()
    nc = _build(meta, total)
    print(f"[kernel] trace {_t.perf_counter()-t0:.1f}s", flush=True)

    in_maps = []
    for k in range(NC):
        pc = per_core[k]
        in_maps.append({
            "w_exp": np.asarray(pc["w_exp"]),
            "idx1": pc["idx1"],
            "idx3": pc["idx3"],
            "idx_bins": pc["idx_bins"],
            "extf": pc["extf"],
            "s_all0": np.asarray(meta["s_all0"]),
        })

    t0 = _t.perf_counter()
    res = run_bass_kernel_spmd(nc, in_maps, core_ids=list(range(NC)))
    dt = _t.perf_counter() - t0
    print(f"[kernel] run(compile+exec cold) {dt:.2f}s", flush=True)

    t0 = _t.perf_counter()
    res = run_bass_kernel_spmd(nc, in_maps, core_ids=list(range(NC)))
    dt = _t.perf_counter() - t0
    LAST_RUN_NS = int(dt * 1e9)
    print(f"[kernel] run(warm) {dt:.3f}s ({dt/total*1e3:.2f} ms/step)",
          flush=True)

    out = np.empty((n_steps, N_NODES), dtype=np.float32)
    for k in range(NC):
        rk = res.results[k]["raster"][equilibration_steps:]  # [n, 128, COLS]
        flat = rk.transpose(0, 2, 1).reshape(n_steps, 128 * COLS)
        out[:, NCD * k:NCD * (k + 1)] = flat[:, :NCD].astype(np.float32)
    return out


# revision 7
# speedup vs baseline: 57.4799x; 43.1960x over previous
"""Trainium2 Bass kernel for nn_AbstractModel_70394513981830 (spiking network).

Per step: s' = sigmoid(segment_sum(W * s[src], dst) - 1), 300 sequential steps,
last 200 recorded. 8 NeuronCores, dst-sharded (NC k owns dsts [12500k,12500k+12500)).

Single fused kernel launch runs all 300 steps (unrolled; one straight-line
AllGather per step). Per-step pipeline on each NC (its 800k edges):

1. s_all [128, 848] bf16 holds the full state: node n = 12500*k + d lives at
   partition d%128, column 106*k + d//128; columns 98..105 of each sender block
   are extension copies of "overloaded" nodes (see below).
2. Expansion multiply (DVE): per-edge products P1[p, occ*848 + j] =
   W_exp[p, occ*848 + j] * s_all[p, j] via a stride-0 broadcast view. Each
   node-column j has a run of R=16 occurrence slots; a (node, receiver) pair
   with >16 in-edges spills into the node's extension column (16 more).
3. Route each product to the partition owning its dst in 3 stages:
   a. local_scatter (per-partition idxs) P1 -> R1 at position 128*b + p_dst
      (b = per (src-partition, dst-partition) bucket fill), in windows of 2047.
   b. full 128x128 block transposes of R1: DVE stream_transpose (32x32) + 16
      block-grid DMAs => R2[p_dst, 128*b + q_src] = R1[q_src, 128*b + p_dst].
   c. local_scatter R2 -> R3: per-partition dst runs, padded to multiples of 8.
4. reduce8 (f32) -> chunk sums; bf16; local_scatter chunks into per-dst
   16-slot bins; reduce16 -> per-dst sums [128, 98] f32; sigmoid(x-1) -> bf16.
5. local_scatter fills the 8 extension columns; shard [128, 106] -> DRAM;
   AllGather; 8 DMAs rebuild s_all; raster row written per step.
"""

import numpy as np

LAST_RUN_NS = 0

N_NODES = 100_000
N_EDGES = 6_400_000
THETA = 1.0
NC = 8
NCD = N_NODES // NC          # 12500 dsts / nodes per NC
COLS = 98                    # ceil(12500 / 128) node columns per NC
EXT = 12                     # extension columns per NC
CPN = COLS + EXT             # 106 columns per NC block in s_all
NODECOLS = NC * CPN          # 848
R = 16                       # occurrence slots per node column
M1 = R * NODECOLS            # 13568 expansion slots per partition
WIN = 2046                   # local_scatter out window size (even)
BINS = 16                    # chunk bins per dst
MAXRANK = 98                 # dst ranks per partition


def _group_rank(keys):
    """Rank of each element within its equal-key group (keys int64 1-D)."""
    order = np.argsort(keys, kind="stable")
    sk = keys[order]
    first = np.r_[0, np.flatnonzero(np.diff(sk)) + 1]
    grp_start = np.zeros(len(sk), dtype=np.int64)
    grp_start[first] = first
    np.maximum.accumulate(grp_start, out=grp_start)
    rank_sorted = np.arange(len(sk)) - grp_start
    rank = np.empty(len(sk), dtype=np.int64)
    rank[order] = rank_sorted
    return rank


def _preprocess(x, W, edge_index):
    import ml_dtypes

    src = np.asarray(edge_index[0], dtype=np.int64)
    dst = np.asarray(edge_index[1], dtype=np.int64)
    W = np.asarray(W, dtype=np.float32)
    s0 = np.asarray(x, dtype=np.float32).reshape(-1)

    k_r = dst // NCD                       # receiver NC per edge
    ks = src // NCD                        # sender NC per src node
    ds = src % NCD
    q_s = ds % 128                         # src partition
    base_j = CPN * ks + ds // 128          # base node column

    # (node, receiver) in-multiplicity; extension column assignment (global)
    cnt = np.bincount(src * NC + k_r, minlength=N_NODES * NC).reshape(N_NODES, NC)
    assert cnt.max() <= 2 * R, f"node multiplicity {cnt.max()} > {2*R}"
    over = (cnt > R).any(axis=1)           # nodes needing an extension column
    node_ks = np.arange(N_NODES) // NCD
    node_q = (np.arange(N_NODES) % NCD) % 128
    ext_col = np.full(N_NODES, -1, dtype=np.int64)
    ext_slot = np.full(N_NODES, -1, dtype=np.int64)
    ov_idx = np.flatnonzero(over)
    slot = _group_rank(node_ks[ov_idx] * 128 + node_q[ov_idx])
    assert len(slot) == 0 or slot.max() < EXT, \
        f"extension slots per (sender,partition) exceed {EXT}: {slot.max()+1}"
    ext_slot[ov_idx] = slot
    ext_col[ov_idx] = CPN * node_ks[ov_idx] + COLS + slot

    per_core = []
    B_all, L3_all = [], []
    cores_tmp = []
    for k in range(NC):
        sel = np.flatnonzero(k_r == k)
        e_src = src[sel]
        e_w = W[sel]
        dl = dst[sel] - NCD * k
        p_d = dl % 128
        r_d = dl // 128
        eq = q_s[sel]

        occ = _group_rank(e_src)
        use_ext = occ >= R
        j = np.where(use_ext, ext_col[e_src], base_j[sel])
        occ_slot = np.where(use_ext, occ - R, occ)
        assert (j >= 0).all() and (occ_slot < R).all()
        m1 = occ_slot * NODECOLS + j       # P1 slot (partition eq)

        b = _group_rank(eq * 128 + p_d)    # bucket fill (q_src, p_dst)
        B = int(b.max()) + 1

        # dst runs (per dst-partition), padded to multiples of 8
        deg = np.bincount(p_d * MAXRANK + r_d,
                          minlength=128 * MAXRANK).reshape(128, MAXRANK)
        runlen = (deg + 7) // 8 * 8
        starts = np.zeros_like(runlen)
        starts[:, 1:] = np.cumsum(runlen, axis=1)[:, :-1]
        L3 = int((starts[:, -1] + runlen[:, -1]).max())
        within = _group_rank(p_d * MAXRANK + r_d)
        slot3 = starts[p_d, r_d] + within

        cores_tmp.append(dict(
            e_src=e_src, e_w=e_w, p_d=p_d, r_d=r_d, eq=eq, m1=m1, b=b,
            slot3=slot3, deg=deg, runlen=runlen, starts=starts,
        ))
        B_all.append(B)
        L3_all.append(L3)

    B = max(B_all)
    M2 = 128 * B                           # routed-layout size
    W1 = -(-M2 // WIN)                     # stage-a windows
    L3 = -(-max(L3_all) // 16) * 16        # multiple of 16 (even NCHUNK)
    W3 = -(-L3 // WIN)
    NCHUNK = L3 // 8
    assert M1 % 2 == 0 and M2 % 2 == 0 and NCHUNK % 2 == 0

    for k in range(NC):
        t = cores_tmp[k]
        eq, m1, p_d, b = t["eq"], t["m1"], t["p_d"], t["b"]
        r1pos = 128 * b + p_d
        idx1 = np.full((W1, 128, M1), -1, dtype=np.int16)
        w = r1pos // WIN
        idx1[w, eq, m1] = (r1pos - w * WIN).astype(np.int16)

        m2 = 128 * b + eq                  # position after transpose (part p_d)
        slot3 = t["slot3"]
        idx3 = np.full((W3, 128, M2), -1, dtype=np.int16)
        w = slot3 // WIN
        idx3[w, p_d, m2] = (slot3 - w * WIN).astype(np.int16)

        # chunk -> bin mapping
        runlen, starts = t["runlen"], t["starts"]
        idx_bins = np.full((128, NCHUNK), -1, dtype=np.int16)
        for p in range(128):
            for r in range(MAXRANK):
                rl = runlen[p, r]
                if rl == 0:
                    continue
                c0 = starts[p, r] // 8
                nch = rl // 8
                assert nch <= BINS
                idx_bins[p, c0:c0 + nch] = np.arange(
                    BINS * r, BINS * r + nch, dtype=np.int16)

        w_exp = np.zeros((128, M1), dtype=ml_dtypes.bfloat16)
        w_exp[eq, m1] = t["e_w"].astype(ml_dtypes.bfloat16)

        # sender-side extension fill: this core's own overloaded nodes
        extf = np.full((128, COLS), -1, dtype=np.int16)
        own = np.arange(NCD * k, NCD * (k + 1))
        ov = over[own]
        d_own = np.arange(NCD)[ov]
        extf[d_own % 128, d_own // 128] = ext_slot[own[ov]].astype(np.int16)

        per_core.append(dict(w_exp=w_exp, idx1=idx1, idx3=idx3,
                             idx_bins=idx_bins, extf=extf))

    # initial s_all (same for all cores)
    s_pad = np.zeros(128 * NODECOLS, dtype=np.float32).reshape(128, NODECOLS)
    n = np.arange(N_NODES)
    s_pad[(n % NCD) % 128, CPN * (n // NCD) + (n % NCD) // 128] = s0
    s_pad[node_q[ov_idx], ext_col[ov_idx]] = s0[ov_idx]
    s_all0 = s_pad.astype(ml_dtypes.bfloat16)

    meta = dict(B=B, M2=M2, W1=W1, W3=W3, L3=L3, NCHUNK=NCHUNK, s_all0=s_all0)
    return meta, per_core


def _np_step(meta, per_core, s_all):
    """Numpy simulation of one device step. s_all [128, 848] bf16 (all cores
    share it). Returns (new s_all bf16, shards list of [128, COLS] bf16)."""
    import ml_dtypes

    B, M2, W1, W3 = meta["B"], meta["M2"], meta["W1"], meta["W3"]
    L3, NCHUNK = meta["L3"], meta["NCHUNK"]
    shards = []
    for k in range(NC):
        pc = per_core[k]
        s_view = np.tile(s_all[:, None, :], (1, R, 1)).reshape(128, M1)
        p1 = (pc["w_exp"].astype(np.float32) * s_view.astype(np.float32)
              ).astype(ml_dtypes.bfloat16)
        r1 = np.zeros((128, W1 * WIN), dtype=ml_dtypes.bfloat16)
        for w in range(W1):
            ii = pc["idx1"][w]
            pmask, mmask = np.nonzero(ii >= 0)
            r1[pmask, w * WIN + ii[pmask, mmask]] = p1[pmask, mmask]
        r2 = np.zeros((128, M2), dtype=ml_dtypes.bfloat16)
        blk = r1[:, :M2].reshape(128, B, 128)
        r2 = blk.transpose(2, 1, 0).reshape(128, M2)
        r3 = np.zeros((128, L3), dtype=ml_dtypes.bfloat16)
        for w in range(W3):
            ii = pc["idx3"][w]
            pmask, mmask = np.nonzero(ii >= 0)
            r3[pmask, w * WIN + ii[pmask, mmask]] = r2[pmask, mmask]
        chunks = r3.astype(np.float32).reshape(128, NCHUNK, 8).sum(-1)
        chunks_bf = chunks.astype(ml_dtypes.bfloat16)
        bins = np.zeros((128, MAXRANK * BINS), dtype=ml_dtypes.bfloat16)
        ib = pc["idx_bins"]
        pmask, cmask = np.nonzero(ib >= 0)
        bins[pmask, ib[pmask, cmask]] = chunks_bf[pmask, cmask]
        sums = bins.astype(np.float32).reshape(128, MAXRANK, BINS).sum(-1)
        snew = (1.0 / (1.0 + np.exp(-(sums - THETA)))).astype(ml_dtypes.bfloat16)
        shards.append(snew)

    new_sall = np.zeros_like(s_all)
    for k in range(NC):
        pc = per_core[k]
        blkk = np.zeros((128, CPN), dtype=ml_dtypes.bfloat16)
        blkk[:, :COLS] = shards[k]
        ef = pc["extf"]
        pmask, cmask = np.nonzero(ef >= 0)
        blkk[pmask, COLS + ef[pmask, cmask]] = shards[k][pmask, cmask]
        new_sall[:, CPN * k:CPN * (k + 1)] = blkk
    return new_sall, shards


def _sall_to_s(s_all):
    """Extract the flat [N_NODES] f32 state from an s_all table."""
    n = np.arange(N_NODES)
    return s_all[(n % NCD) % 128,
                 CPN * (n // NCD) + (n % NCD) // 128].astype(np.float32)


def _build(meta, steps):
    """Build the unrolled bass program (shared by all 8 cores)."""
    import concourse.bass as bass
    import concourse.bacc as bacc
    import concourse.mybir as mybir
    import concourse.tile as tile

    f32 = mybir.dt.float32
    bf16 = mybir.dt.bfloat16
    i16 = mybir.dt.int16
    AF = mybir.ActivationFunctionType
    ALU = mybir.AluOpType
    AX = mybir.AxisListType

    B, M2, W1, W3 = meta["B"], meta["M2"], meta["W1"], meta["W3"]
    L3, NCHUNK = meta["L3"], meta["NCHUNK"]

    nc = bacc.Bacc(num_devices=NC)
    g_wexp = nc.dram_tensor("w_exp", [128, M1], bf16, kind="ExternalInput")
    g_idx1 = nc.dram_tensor("idx1", [W1, 128, M1], i16, kind="ExternalInput")
    g_idx3 = nc.dram_tensor("idx3", [W3, 128, M2], i16, kind="ExternalInput")
    g_bins = nc.dram_tensor("idx_bins", [128, NCHUNK], i16, kind="ExternalInput")
    g_extf = nc.dram_tensor("extf", [128, COLS], i16, kind="ExternalInput")
    g_sall0 = nc.dram_tensor("s_all0", [128, NODECOLS], bf16, kind="ExternalInput")
    raster = nc.dram_tensor("raster", [steps, 128, COLS], bf16,
                            kind="ExternalOutput")
    cc_in = nc.dram_tensor("cc_in", [128, CPN], bf16)
    cc_out = nc.dram_tensor("cc_out", [NC * 128, CPN], bf16, addr_space="Shared")

    with tile.TileContext(nc) as tc:
        with tc.tile_pool(name="res", bufs=1) as res, \
             tc.tile_pool(name="big", bufs=1) as big, \
             tc.tile_pool(name="idx", bufs=2) as idxp:
            t_wexp = res.tile([128, M1], bf16)
            t_sall = res.tile([128, NODECOLS], bf16)
            t_bins_i = res.tile([128, NCHUNK], i16)
            t_extf = res.tile([128, COLS], i16)
            t_bias = res.tile([128, 1], f32)
            nc.vector.memset(t_bias[:], -THETA)
            nc.sync.dma_start(t_wexp[:], g_wexp[:])
            nc.sync.dma_start(t_sall[:], g_sall0[:])
            nc.sync.dma_start(t_bins_i[:], g_bins[:])
            nc.sync.dma_start(t_extf[:], g_extf[:])

            for t in range(steps):
                # 1. expansion multiply
                t_p1 = big.tile([128, M1], bf16, tag="p1t1")
                s_b = t_sall[:].unsqueeze(1).broadcast_to([128, R, NODECOLS])
                nc.vector.tensor_tensor(
                    t_p1[:].rearrange("p (o j) -> p o j", o=R),
                    t_wexp[:].rearrange("p (o j) -> p o j", o=R),
                    s_b, ALU.mult)

                # 2a. route scatter 1 (P1 -> R1)
                t_r1 = big.tile([128, W1 * WIN], bf16, tag="r1r2")
                for w in range(W1):
                    t_i1 = idxp.tile([128, M1], i16, tag="idx")
                    nc.sync.dma_start(t_i1[:], g_idx1[w])
                    nc.gpsimd.local_scatter(
                        t_r1[:, w * WIN:(w + 1) * WIN], t_p1[:], t_i1[:],
                        channels=128, num_elems=WIN, num_idxs=M1)

                # 2b. 128-block transposes
                t_t1 = big.tile([128, M2], bf16, tag="p1t1")
                nc.vector.transpose(t_t1[:], t_r1[:, :M2])
                t_r2 = big.tile([128, W1 * WIN], bf16, tag="r1r2")
                for a in range(4):
                    for d in range(4):
                        nc.sync.dma_start(
                            t_r2[32 * a:32 * a + 32, :M2]
                            .rearrange("p (b d w) -> p b d w", d=4, w=32)
                            [:, :, d, :],
                            t_t1[32 * d:32 * d + 32, :]
                            .rearrange("p (b a w) -> p b a w", a=4, w=32)
                            [:, :, a, :])

                # 2c. route scatter 2 (R2 -> R3 dst runs)
                t_r3 = big.tile([128, L3], bf16, tag="r3")
                for w in range(W3):
                    t_i3 = idxp.tile([128, M2], i16, tag="idx")
                    nc.sync.dma_start(t_i3[:], g_idx3[w])
                    nc.gpsimd.local_scatter(
                        t_r3[:, w * WIN:min((w + 1) * WIN, L3)], t_r2[:, :M2],
                        t_i3[:], channels=128,
                        num_elems=min(WIN, L3 - w * WIN), num_idxs=M2)

                # 3. segment sums
                t_ch = big.tile([128, NCHUNK], f32, tag="ch")
                nc.vector.tensor_reduce(
                    t_ch[:], t_r3[:].rearrange("p (c e) -> p c e", e=8),
                    axis=AX.X, op=ALU.add)
                t_chb = big.tile([128, NCHUNK], bf16, tag="chb")
                nc.vector.tensor_copy(t_chb[:], t_ch[:])
                t_bins = big.tile([128, MAXRANK * BINS], bf16, tag="bins")
                nc.gpsimd.local_scatter(
                    t_bins[:], t_chb[:], t_bins_i[:],
                    channels=128, num_elems=MAXRANK * BINS, num_idxs=NCHUNK)
                t_sum = big.tile([128, MAXRANK], f32, tag="sum")
                nc.vector.tensor_reduce(
                    t_sum[:], t_bins[:].rearrange("p (r e) -> p r e", e=BINS),
                    axis=AX.X, op=ALU.add)

                # 4. sigmoid + extension fill -> shard
                t_shard = big.tile([128, CPN], bf16, tag="shard")
                nc.scalar.activation(t_shard[:, :COLS], t_sum[:], AF.Sigmoid,
                                     bias=t_bias[:])
                nc.gpsimd.local_scatter(
                    t_shard[:, COLS:CPN], t_shard[:, :COLS], t_extf[:],
                    channels=128, num_elems=EXT, num_idxs=COLS)

                # 5. raster + exchange + s_all rebuild
                nc.sync.dma_start(raster[t], t_shard[:, :COLS])
                nc.sync.dma_start(cc_in[:], t_shard[:])
                nc.gpsimd.collective_compute(
                    "AllGather", ALU.bypass,
                    replica_groups=[list(range(NC))],
                    ins=[cc_in[:]], outs=[cc_out[:]])
                for k in range(NC):
                    nc.sync.dma_start(
                        t_sall[:, CPN * k:CPN * (k + 1)],
                        cc_out[128 * k:128 * (k + 1), :])
    nc.finalize()
    return nc


def _make_runner(nc, in_maps):
    """Compile the SPMD program once; return (run_fn, fetch) where run_fn()
    executes with device-resident inputs (no host->device transfer)."""
    import jax
    import jax.numpy as jnp
    import concourse.mybir as mybir
    from jax.sharding import Mesh, NamedSharding, PartitionSpec as P
    from jax.experimental.shard_map import shard_map
    from concourse import bass2jax

    bass2jax.install_neuronx_cc_hook()

    in_names, out_names, out_avals, zero_shapes = [], [], [], []
    partition_name = (nc.partition_id_tensor.name
                      if nc.partition_id_tensor else None)
    for alloc in nc.m.functions[0].allocations:
        if not isinstance(alloc, mybir.MemoryLocationSet):
            continue
        name = alloc.memorylocations[0].name
        if alloc.kind == "ExternalInput" and name != partition_name:
            in_names.append(name)
        elif alloc.kind == "ExternalOutput":
            out_names.append(name)
            shape = tuple(alloc.tensor_shape)
            dtype = mybir.dt.np(alloc.dtype)
            out_avals.append(jax.core.ShapedArray(shape, dtype))
            zero_shapes.append((shape, dtype))
    n_params = len(in_names)
    n_outs = len(out_names)
    all_in_names = list(in_names) + list(out_names)
    if partition_name is not None:
        all_in_names.append(partition_name)
    donate = tuple(range(n_params, n_params + n_outs))

    def _body(*args):
        operands = list(args)
        if partition_name is not None:
            operands.append(bass2jax.partition_id_tensor())
        return tuple(bass2jax._bass_exec_p.bind(
            *operands,
            out_avals=tuple(out_avals),
            in_names=tuple(all_in_names),
            out_names=tuple(out_names),
            lowering_input_output_aliases=(),
            sim_require_finite=True,
            sim_require_nnan=True,
            nc=nc,
        ))

    devices = jax.devices()[:NC]
    mesh = Mesh(np.asarray(devices), ("core",))
    sharded = jax.jit(
        shard_map(_body, mesh=mesh,
                  in_specs=(P("core"),) * (n_params + n_outs),
                  out_specs=(P("core"),) * n_outs,
                  check_rep=False),
        donate_argnums=donate, keep_unused=True,
    )
    sh = NamedSharding(mesh, P("core"))
    dev_in = [
        jax.device_put(
            np.concatenate([np.asarray(in_maps[c][name])
                            for c in range(NC)], axis=0), sh)
        for name in in_names
    ]
    zero_fns = [
        jax.jit(lambda s=shape, d=dtype: jnp.zeros((NC * s[0],) + s[1:], d),
                out_shardings=sh)
        for shape, dtype in zero_shapes
    ]

    def run_fn():
        zeros = [zf() for zf in zero_fns]
        jax.block_until_ready(zeros)
        import time as _t
        t0 = _t.perf_counter()
        outs = sharded(*dev_in, *zeros)
        jax.block_until_ready(outs)
        return _t.perf_counter() - t0, outs

    def fetch(outs):
        res = []
        for c in range(NC):
            res.append({
                name: np.asarray(outs[i]).reshape(NC, *out_avals[i].shape)[c]
                for i, name in enumerate(out_names)})
        return res

    return run_fn, fetch


def kernel(x, W, edge_index, n_steps, equilibration_steps):
    import time as _t

    global LAST_RUN_NS
    n_steps = int(n_steps)
    equilibration_steps = int(equilibration_steps)
    total = n_steps + equilibration_steps

    t0 = _t.perf_counter()
    meta, per_core = _preprocess(x, W, edge_index)
    print(f"[kernel] preprocess {_t.perf_counter()-t0:.1f}s "
          f"B={meta['B']} W1={meta['W1']} L3={meta['L3']}", flush=True)

    t0 = _t.perf_counter()
    nc = _build(meta, total)
    print(f"[kernel] trace {_t.perf_counter()-t0:.1f}s", flush=True)

    in_maps = []
    for k in range(NC):
        pc = per_core[k]
        in_maps.append({
            "w_exp": np.asarray(pc["w_exp"]),
            "idx1": pc["idx1"],
            "idx3": pc["idx3"],
            "idx_bins": pc["idx_bins"],
            "extf": pc["extf"],
            "s_all0": np.asarray(meta["s_all0"]),
        })

    t0 = _t.perf_counter()
    run_fn, fetch = _make_runner(nc, in_maps)
    dt_cold, outs = run_fn()
    print(f"[kernel] compile+first-exec {_t.perf_counter()-t0:.2f}s",
          flush=True)

    dt, outs = run_fn()
    LAST_RUN_NS = int(dt * 1e9)
    print(f"[kernel] run(warm) {dt:.3f}s ({dt/total*1e3:.2f} ms/step)",
          flush=True)

    results = fetch(outs)
    out = np.empty((n_steps, N_NODES), dtype=np.float32)
    for k in range(NC):
        rk = results[k]["raster"][equilibration_steps:]  # [n, 128, COLS]
        flat = rk.transpose(0, 2, 1).reshape(n_steps, 128 * COLS)
        out[:, NCD * k:NCD * (k + 1)] = flat[:, :NCD].astype(np.float32)
    return out
